# revision 61
# baseline (speedup 1.0000x reference)
"""YOLOv1 loss kernel for Trainium2, 8-core data-parallel, bf16 pipeline.

Strategy: shard batch (8192) across 8 cores (1024 rows each). Host converts
inputs to bf16 (labels obj channel converted equality-preserving so l4 == 1.0
stays exact) and repacks channels per-row so every multi-channel device op is
one contiguous instruction:

  pred row (30 ch):  [0,1,5,6 | 2,3,7,8 | 4,9 | 10..29]
  label row (35 ch): [0,1,0,1 | 2,3,2,3 | 5,6 | 2,3,7,8 | 4 | 10..29]

Each core streams its shard in uneven chunks (g units of 128 rows, layout
[128, g, ch, 49]). Per chunk the whole loss reduces to ONE Scalar-engine
Square+accumulate over a packed masked buffer mball[g, 32, 49]:
  slots 0:8   sqrt(5*om_b) * (coor diffs: dx, dy, sqrt-w, sqrt-h per box)
  slots 8:10  sqrt(mA/mB) * (conf - iou) per box
  slots 10:12 sqrt(0.5*(1-obj)) * (p4, p9)
  slots 12:32 obj * (pred_cls - label_cls)
where om_u = obj*use1, om_nu = obj*(1-use1), mA = om_u + 0.5*om_nu,
mB = om_nu + 0.5*om_u. sum(mball^2) == chunk loss contribution exactly.

IoU runs in 7x-scaled units: ov7 = max(min(7*min(w), 3.5*(wp+wl) - |dc|), 0),
ints49 = ov7w*ov7h, u49 = 49*(area_p + area_g) - ints49, iou = ints49/u49.

All sqrt-mask values are linear in use1 (e.g. sqrt(mA) = 0.7071 + 0.2929*u,
sqrt(5*om_u) = sqrt(5)*obj*u), so the mask vector rm[5] is built with plain
mult/add ops - no activation-engine involvement on the mask path.

Engine split: DVE does diffs/IoU (tensor_tensor 2x bf16, tensor_scalar 4x)
with the masked-multiply wave lagged one chunk behind (software pipelining);
Pool (gpsimd) computes obj, the rm vector, 8 of 20 cls channels and the
masked conf/q multiplies; ACT does the w/h sqrts and one Square+accum_out
per chunk (lagged two chunks so sqrts never queue behind it). Every
chunk's DMA is split into an IoU-channel head and a cls tail so IoU work
starts before the cls channels land; the last chunk computes its own mask
vector on DVE to shorten the tail.
Out: acc[128, NCHUNK] fp32 per core, summed on host in fp64.
"""

import sys

import numpy as np

for _p in ("/opt/trn_rl_repo", "/root/.axon_site/_ro/trn_rl_repo"):
    if _p not in sys.path:
        sys.path.insert(0, _p)

import concourse.bass as bass
import concourse.mybir as mybir
from concourse.bass_utils import run_bass_kernel_spmd

F32 = mybir.dt.float32
BF16 = mybir.dt.bfloat16
U16 = mybir.dt.uint16
Alu = mybir.AluOpType
Act = mybir.ActivationFunctionType

B_TOTAL = 8192
NCORES = 8
B_CORE = B_TOTAL // NCORES  # 1024
P = 128
C = 30
K = 49
CP = 30   # repacked pred channels
CL = 35   # repacked label channels
ROWP = CP * K
ROWL = CL * K

# host channel permutations
PP_IDX = [0, 1, 5, 6, 2, 3, 7, 8, 4, 9] + list(range(10, 30))
LL_IDX = [0, 1, 0, 1, 2, 3, 2, 3, 5, 6, 2, 3, 7, 8, 4] + list(range(10, 30))

CHUNKS = (2, 3, 2, 1)


def build_nc(chunks=CHUNKS):
    assert sum(chunks) * P == B_CORE
    nchunk = len(chunks)
    maxg = max(chunks)
    nc = bass.Bass()
    pred = nc.declare_dram_parameter("pred", [B_CORE, ROWP], BF16, isOutput=False)
    labels = nc.declare_dram_parameter("labels", [B_CORE, ROWL], BF16,
                                       isOutput=False)
    out = nc.declare_dram_parameter("out", [P, nchunk], F32, isOutput=True)

    from contextlib import ExitStack

    _ctr = [0]
    es = ExitStack()

    def sb(shape, dt=BF16):
        _ctr[0] += 1
        return es.enter_context(nc.sbuf_tensor(f"t{_ctr[0]}", shape, dt))

    with es:
        pt = [sb([P, maxg, CP, K]) for _ in range(2)]
        lt = [sb([P, maxg, CL, K]) for _ in range(2)]
        # dxyb slots: 0:2 b2-iou-xy, 2:4 b1-xy, 4:6 b1-sqrt, 6:8 b2-xy, 8:10 b2-sqrt
        dxyb = [sb([P, maxg, 10, K]) for _ in range(2)]
        adb = [sb([P, maxg, 4, K]) for _ in range(2)]     # |b2Ix,b2Iy,b1x,b1y|
        swh = [sb([P, maxg, 2, 2, K]) for _ in range(2)]
        s35 = [sb([P, maxg, 2, 2, K]) for _ in range(2)]
        mwh = [sb([P, maxg, 2, 2, K]) for _ in range(2)]
        mwh7 = [sb([P, maxg, 2, 2, K]) for _ in range(2)]
        ta = [sb([P, maxg, 2, 2, K]) for _ in range(2)]
        ov = [sb([P, maxg, 2, 2, K]) for _ in range(2)]
        cl = [sb([P, maxg, 2, 2, K]) for _ in range(2)]
        ints = [sb([P, maxg, 2, K]) for _ in range(2)]
        apw = [sb([P, maxg, 2, K]) for _ in range(2)]
        agb = [sb([P, maxg, 1, K]) for _ in range(2)]
        sa = [sb([P, maxg, 2, K]) for _ in range(2)]
        sa49 = [sb([P, maxg, 2, K]) for _ in range(2)]
        u49 = [sb([P, maxg, 2, K]) for _ in range(2)]
        rcp = [sb([P, maxg, 2, K]) for _ in range(2)]
        iou = [sb([P, maxg, 2, K]) for _ in range(2)]
        crx = [sb([P, maxg, 2, K]) for _ in range(2)]
        tq = [sb([P, maxg, 5, K]) for _ in range(2)]  # u, then 4 factors
        dconf = [sb([P, maxg, 2, K]) for _ in range(2)]
        dcls = [sb([P, maxg, 12, K]) for _ in range(2)]
        dclsp = [sb([P, maxg, 8, K]) for _ in range(2)]
        sqp = [sb([P, maxg, 2, 2, K]) for _ in range(2)]
        sql = [sb([P, maxg, 2, 2, K]) for _ in range(2)]
        objm = [sb([P, maxg, K]) for _ in range(2)]
        objd = sb([P, maxg, K])
        dclsL = sb([P, maxg, 20, K])
        rm = [sb([P, maxg, 5, K]) for _ in range(2)]
        mball = [sb([P, maxg, 32, K]) for _ in range(2)]
        junk32 = [sb([P, maxg, 32, K]) for _ in range(2)]
        acc = es.enter_context(nc.sbuf_tensor("acc", [P, nchunk], F32))

        dsemA = es.enter_context(nc.semaphore("dsemA"))
        dsemB = es.enter_context(nc.semaphore("dsemB"))
        dsems = [dsemA, dsemB]
        u_done = es.enter_context(nc.semaphore("u_done"))
        sqrt_done = es.enter_context(nc.semaphore("sqrt_done"))
        rm_done = es.enter_context(nc.semaphore("rm_done"))
        mball_dve = es.enter_context(nc.semaphore("mball_dve"))
        mball_pool = es.enter_context(nc.semaphore("mball_pool"))
        acc_done = es.enter_context(nc.semaphore("acc_done"))
        tfree_dve = es.enter_context(nc.semaphore("tfree_dve"))
        area_done = es.enter_context(nc.semaphore("area_done"))
        pmwh = es.enter_context(nc.semaphore("pmwh"))
        dsemTA = es.enter_context(nc.semaphore("dsemTA"))
        dsemTB = es.enter_context(nc.semaphore("dsemTB"))
        dsemTs = [dsemTA, dsemTB]
        block = es.enter_context(nc.Block())

        offs = [0]
        for g in chunks:
            offs.append(offs[-1] + g * P)

        # dsem thresholds: chunk 0 is split into head (iou ch) + tail (cls ch)
        head_v = {}
        tail_v = {}
        _dv = [0, 0]
        for i in range(nchunk):
            s = i % 2
            head_v[i] = tail_v[i] = _dv[s] + 32
            _dv[s] += 32

        @block.sync
        def _(sync):
            for i, g in enumerate(chunks):
                s = i % 2
                if i >= 2:
                    sync.wait_ge(sqrt_done, i - 1)
                    sync.wait_ge(mball_pool, i - 1)
                    sync.wait_ge(tfree_dve, i - 1)
                rows = slice(offs[i], offs[i + 1])
                sync.dma_start(
                    out=pt[s][:, 0:g, 0:10, :].rearrange(
                        "p g c k -> p g (c k)"),
                    in_=pred[rows, 0:10 * K].rearrange(
                        "(g p) d -> p g d", p=P),
                ).then_inc(dsems[s], 16)
                sync.dma_start(
                    out=lt[s][:, 0:g, 0:15, :].rearrange(
                        "p g c k -> p g (c k)"),
                    in_=labels[rows, 0:15 * K].rearrange(
                        "(g p) d -> p g d", p=P),
                ).then_inc(dsems[s], 16)
                sync.dma_start(
                    out=pt[s][:, 0:g, 10:30, :].rearrange(
                        "p g c k -> p g (c k)"),
                    in_=pred[rows, 10 * K:].rearrange(
                        "(g p) d -> p g d", p=P),
                ).then_inc(dsemTs[s], 16)
                sync.dma_start(
                    out=lt[s][:, 0:g, 15:35, :].rearrange(
                        "p g c k -> p g (c k)"),
                    in_=labels[rows, 15 * K:].rearrange(
                        "(g p) d -> p g d", p=P),
                ).then_inc(dsemTs[s], 16)
            sync.wait_ge(acc_done, nchunk)
            sync.dma_start(out=out[:], in_=acc[:]).then_inc(dsemA, 16)
            sync.wait_ge(dsemA, _dv[0] + 16)

        @block.gpsimd
        def _(gp):
            for i, g in enumerate(chunks):
                if i == nchunk - 1:
                    continue
                s = i % 2
                gp.wait_ge(dsems[s], head_v[i])
                if i >= 2:
                    gp.wait_ge(mball_dve, i - 1)
                gp.tensor_scalar(objm[s][:, 0:g], lt[s][:, 0:g, 14:15, :],
                                 1.0, None, Alu.is_equal)
                gp.wait_ge(dsemTs[s], tail_v[i])
                gp.tensor_tensor(dclsp[s][:, 0:g], pt[s][:, 0:g, 22:30, :],
                                 lt[s][:, 0:g, 27:35, :], Alu.subtract)
                gp.drain()
                gp.wait_ge(u_done, i + 1)
                gp.tensor_scalar(rm[s][:, 0:g, 4:5, :], objm[s][:, 0:g],
                                 -0.70710678, 0.70710678, Alu.mult,
                                 Alu.add)
                gp.tensor_tensor(
                    rm[s][:, 0:g, 0:4, :],
                    objm[s][:, 0:g].unsqueeze(2).broadcast_to(
                        [P, g, 4, K]),
                    tq[s][:, 0:g, 1:5, :], Alu.mult,
                ).then_inc(rm_done, 1)
                gp.drain()
                if i >= 2:
                    gp.wait_ge(acc_done, i - 1)
                gp.tensor_tensor(
                    mball[s][:, 0:g, 24:32, :], dclsp[s][:, 0:g],
                    objm[s][:, 0:g].unsqueeze(2).broadcast_to([P, g, 8, K]),
                    Alu.mult,
                )
                gp.tensor_tensor(
                    mball[s][:, 0:g, 10:12, :], pt[s][:, 0:g, 8:10, :],
                    rm[s][:, 0:g, 4:5, :].broadcast_to([P, g, 2, K]),
                    Alu.mult,
                )
                gp.tensor_tensor(
                    mball[s][:, 0:g, 8:10, :], dconf[s][:, 0:g],
                    rm[s][:, 0:g, 2:4, :], Alu.mult,
                ).then_inc(mball_pool, 1)

        @block.scalar
        def _(act):
            def sq_acc(j):
                sj = j % 2
                gj = chunks[j]
                act.wait_ge(mball_dve, j + 1)
                act.wait_ge(mball_pool, j + 1)
                act.activation(
                    junk32[sj][:, 0:gj].rearrange("p g c k -> p (g c k)"),
                    mball[sj][:, 0:gj].rearrange("p g c k -> p (g c k)"),
                    Act.Square,
                    accum_out=acc[:, j:j + 1],
                ).then_inc(acc_done, 1)

            for i, g in enumerate(chunks):
                s = i % 2
                act.wait_ge(dsems[s], head_v[i])
                act.activation(sqp[s][:, 0:g], pt[s][:, 0:g, 4:8, :], Act.Sqrt)
                act.activation(sql[s][:, 0:g], lt[s][:, 0:g, 10:14, :],
                               Act.Sqrt).then_inc(sqrt_done, 1)
                if i >= 2:
                    sq_acc(i - 2)
            sq_acc(nchunk - 2)
            sq_acc(nchunk - 1)

        @block.vector
        def _(v):
            tt = v.tensor_tensor
            ts = v.tensor_scalar

            def lagged_mults(j):
                sj = j % 2
                gj = chunks[j]
                v.wait_ge(rm_done, j + 1)
                if j >= 2:
                    v.wait_ge(acc_done, j - 1)
                tt(mball[sj][:, 0:gj, 0:4, :], dxyb[sj][:, 0:gj, 2:6, :],
                   rm[sj][:, 0:gj, 0:1, :].broadcast_to([P, gj, 4, K]),
                   Alu.mult)
                tt(mball[sj][:, 0:gj, 4:8, :], dxyb[sj][:, 0:gj, 6:10, :],
                   rm[sj][:, 0:gj, 1:2, :].broadcast_to([P, gj, 4, K]),
                   Alu.mult)
                if j != nchunk - 1:
                    tt(mball[sj][:, 0:gj, 12:24, :], dcls[sj][:, 0:gj],
                       objm[sj][:, 0:gj].unsqueeze(2).broadcast_to(
                           [P, gj, 12, K]),
                       Alu.mult).then_inc(mball_dve, 1)
                else:
                    v.wait_ge(mball_pool, nchunk - 1)
                    tt(mball[sj][:, 0:gj, 10:12, :], pt[sj][:, 0:gj, 8:10, :],
                       rm[sj][:, 0:gj, 4:5, :].broadcast_to([P, gj, 2, K]),
                       Alu.mult)
                    tt(mball[sj][:, 0:gj, 8:10, :], dconf[sj][:, 0:gj],
                       rm[sj][:, 0:gj, 2:4, :],
                       Alu.mult).then_inc(mball_pool, 1)
                    tt(mball[sj][:, 0:gj, 12:32, :], dclsL[:, 0:gj],
                       objd[:, 0:gj].unsqueeze(2).broadcast_to(
                           [P, gj, 20, K]),
                       Alu.mult).then_inc(mball_dve, 1)
                v.drain()

            for i, g in enumerate(chunks):
                s = i % 2
                p, l = pt[s], lt[s]
                if i == nchunk - 1 and i >= 1:
                    lagged_mults(i - 1)
                v.wait_ge(dsems[s], head_v[i])
                # W1: reads only tiles
                tt(dxyb[s][:, 0:g, 2:4, :], p[:, 0:g, 0:2, :],
                   l[:, 0:g, 0:2, :], Alu.subtract)
                tt(dxyb[s][:, 0:g, 0:2, :], p[:, 0:g, 2:4, :],
                   l[:, 0:g, 2:4, :], Alu.subtract)
                tt(dxyb[s][:, 0:g, 6:8, :], p[:, 0:g, 2:4, :],
                   l[:, 0:g, 8:10, :], Alu.subtract)
                tt(swh[s][:, 0:g], p[:, 0:g, 4:8, :], l[:, 0:g, 4:8, :],
                   Alu.add)
                tt(mwh[s][:, 0:g], p[:, 0:g, 4:8, :], l[:, 0:g, 4:8, :],
                   Alu.min)
                tt(apw[s][:, 0:g],
                   p[:, 0:g, 4:8, :].rearrange("p g (b w) k -> p g b w k",
                                               b=2)[:, :, :, 0, :],
                   p[:, 0:g, 4:8, :].rearrange("p g (b w) k -> p g b w k",
                                               b=2)[:, :, :, 1, :],
                   Alu.mult)
                tt(agb[s][:, 0:g], l[:, 0:g, 4:5, :], l[:, 0:g, 5:6, :],
                   Alu.mult)
                if i == nchunk - 1:
                    ts(objd[:, 0:g], l[:, 0:g, 14:15, :], 1.0, None,
                       Alu.is_equal)
                v.wait_ge(dsemTs[s], tail_v[i])
                if i != nchunk - 1:
                    tt(dcls[s][:, 0:g], p[:, 0:g, 10:22, :],
                       l[:, 0:g, 15:27, :], Alu.subtract)
                else:
                    tt(dclsL[:, 0:g], p[:, 0:g, 10:30, :],
                       l[:, 0:g, 15:35, :], Alu.subtract)
                v.drain()
                # W2
                ts(adb[s][:, 0:g, 0:2, :].bitcast(U16),
                   dxyb[s][:, 0:g, 2:4, :].bitcast(U16),
                   0x7FFF, None, Alu.bitwise_and)
                ts(adb[s][:, 0:g, 2:4, :].bitcast(U16),
                   dxyb[s][:, 0:g, 0:2, :].bitcast(U16),
                   0x7FFF, None, Alu.bitwise_and)
                ts(s35[s][:, 0:g], swh[s][:, 0:g], 3.5, None, Alu.mult)
                ts(mwh7[s][:, 0:g], mwh[s][:, 0:g], 7.0, None, Alu.mult)
                tt(sa[s][:, 0:g], apw[s][:, 0:g],
                   agb[s][:, 0:g].broadcast_to([P, g, 2, K]), Alu.add)
                v.drain()
                # W3  (adb is box-reversed relative to s35: flip its view)
                tt(ta[s][:, 0:g], s35[s][:, 0:g],
                   adb[s][:, 0:g].rearrange("p g (a c) k -> p g a c k", a=2),
                   Alu.subtract)
                ts(sa49[s][:, 0:g], sa[s][:, 0:g], 49.0, None, Alu.mult)
                v.drain()
                tt(ov[s][:, 0:g], mwh7[s][:, 0:g], ta[s][:, 0:g], Alu.min)
                v.drain()
                ts(cl[s][:, 0:g], ov[s][:, 0:g], 0.0, None, Alu.max)
                v.drain()
                tt(ints[s][:, 0:g], cl[s][:, 0:g, :, 0:1, :],
                   cl[s][:, 0:g, :, 1:2, :], Alu.mult)
                v.drain()
                tt(u49[s][:, 0:g], sa49[s][:, 0:g], ints[s][:, 0:g],
                   Alu.subtract)
                v.drain()
                with nc.allow_low_precision(reason="bf16 iou tolerated"):
                    v.reciprocal(rcp[s][:, 0:g], u49[s][:, 0:g])
                v.drain()
                tt(iou[s][:, 0:g], ints[s][:, 0:g], rcp[s][:, 0:g], Alu.mult)
                v.drain()
                tt(tq[s][:, 0:g, 0:1, :], iou[s][:, 0:g, 0:1, :],
                   iou[s][:, 0:g, 1:2, :], Alu.is_ge)
                v.drain()
                # dconf; tq = linear sqrt-mask factors of u ; dsq
                tt(dconf[s][:, 0:g], p[:, 0:g, 8:10, :], iou[s][:, 0:g],
                   Alu.subtract)
                ts(tq[s][:, 0:g, 1:2, :], tq[s][:, 0:g, 0:1, :],
                   2.23606798, None, Alu.mult)
                ts(tq[s][:, 0:g, 2:3, :], tq[s][:, 0:g, 0:1, :],
                   -2.23606798, 2.23606798, Alu.mult, Alu.add)
                ts(tq[s][:, 0:g, 3:4, :], tq[s][:, 0:g, 0:1, :],
                   0.29289322, 0.70710678, Alu.mult, Alu.add)
                ts(tq[s][:, 0:g, 4:5, :], tq[s][:, 0:g, 0:1, :],
                   -0.29289322, 1.0, Alu.mult, Alu.add).then_inc(u_done, 1)
                v.wait_ge(sqrt_done, i + 1)
                tt(dxyb[s][:, 0:g, 4:6, :], sqp[s][:, 0:g, 0:1, :, :],
                   sql[s][:, 0:g, 0:1, :, :], Alu.subtract)
                tt(dxyb[s][:, 0:g, 8:10, :], sqp[s][:, 0:g, 1:2, :, :],
                   sql[s][:, 0:g, 1:2, :, :], Alu.subtract)
                v.drain().then_inc(tfree_dve, 1)
                if i == nchunk - 1:
                    ts(rm[s][:, 0:g, 4:5, :], objd[:, 0:g],
                       -0.70710678, 0.70710678, Alu.mult, Alu.add)
                    tt(rm[s][:, 0:g, 0:4, :],
                       objd[:, 0:g].unsqueeze(2).broadcast_to([P, g, 4, K]),
                       tq[s][:, 0:g, 1:5, :],
                       Alu.mult).then_inc(rm_done, 1)
                    v.drain()
                if i >= 1 and i != nchunk - 1:
                    lagged_mults(i - 1)
            lagged_mults(nchunk - 1)

    return nc


_NC_CACHE = {}


def _get_nc():
    if "nc" not in _NC_CACHE:
        _NC_CACHE["nc"] = build_nc()
    return _NC_CACHE["nc"]


def _to_bf16_repack(pred, labels):
    import ml_dtypes

    bf = ml_dtypes.bfloat16
    p = np.ascontiguousarray(pred, dtype=np.float32).reshape(B_TOTAL, C, K)
    l = np.ascontiguousarray(labels, dtype=np.float32).reshape(B_TOTAL, C, K)
    pb = p.astype(bf)
    lb = l.astype(bf)
    # obj channel: keep the ==1.0 test exact under rounding
    l4 = l[:, 4, :]
    lb4 = lb[:, 4, :]
    bad = (l4 != np.float32(1.0)) & (lb4.astype(np.float32) == np.float32(1.0))
    if bad.any():
        lb4[bad] = bf(0.99609375)
        lb[:, 4, :] = lb4
    prp = np.ascontiguousarray(pb[:, PP_IDX, :]).reshape(B_TOTAL, ROWP)
    lrp = np.ascontiguousarray(lb[:, LL_IDX, :]).reshape(B_TOTAL, ROWL)
    return prp, lrp


def run_device(pred, labels, trace=False):
    nc = _get_nc()
    prp, lrp = _to_bf16_repack(pred, labels)
    in_maps = []
    for c in range(NCORES):
        rows = slice(c * B_CORE, (c + 1) * B_CORE)
        in_maps.append({"pred": prp[rows], "labels": lrp[rows]})
    res = run_bass_kernel_spmd(nc, in_maps, list(range(NCORES)), trace=trace)
    total = 0.0
    for c in range(NCORES):
        total += float(res.results[c]["out"].astype(np.float64).sum())
    loss = np.float32(total / B_TOTAL)
    return loss, res


def kernel(pred, labels):
    loss, _ = run_device(pred, labels, trace=False)
    return np.array(loss, dtype=np.float32)


if __name__ == "__main__":
    rng = np.random.default_rng(0)
    p = rng.random((B_TOTAL, C, 7, 7), dtype=np.float32)
    l = rng.random((B_TOTAL, C, 7, 7), dtype=np.float32)
    l[:, 4] = (rng.random((B_TOTAL, 7, 7)) < 0.3).astype(np.float32)
    print(kernel(p, l))
